# revision 3
# baseline (speedup 1.0000x reference)
"""Trainium2 Bass kernel for the binarized 2-layer MLP (nn_FC_small), v2.

Network (reference semantics):
    h  = sign(x) @ sign(W1).T            # [B, 512], B = 65536, in = 768
    h  = batchnorm(h, g1, b1)            # training-mode, full-batch stats
    h  = clip(h, -1, 1)                  # hardtanh (sign-preserving)
    o  = sign(h) @ sign(W2).T            # [B, 10]
    o  = batchnorm(o, g2, b2)

Identities: sign(clip(z)) == sign(z); with g>0,
sign((h-mu)*r*g + b) == sign(h - T), T = mu - b/(r*g).  Per-k-chunk code
conventions ({0,1} via DVE is_gt vs {-1,1} via ACT sign) are absorbed
into the stationary weight scaling; the resulting batch-independent
column shift is absorbed by the batch statistics, so thresholds live in
raw-PSUM units and no constants are materialized.

v2 changes vs v1 (trace-driven):
  * x loaded fp32 via HWDGE (nc.sync) -- the v1 SWDGE cast-DMA capped at
    ~½ HBM rate (cast engine reads 4B writes 2B); fp32 transposes on PE.
  * per-512-chunk interleave of transposes and DR matmuls keeps HAM warm
    (transpose-mode does not count as PE activity for the clock gate).
  * stats AllGather split: chunks 0-11 gathered while 12-15 compute;
    only the small tail gather is exposed.
  * phase B: all-sign fp8 codes (ACT w/ per-partition threshold bias),
    fp8 DoubleRow mm2, per-2-chunk pipeline, stats on 4-chunk groups.

Sharding: data-parallel over batch across 8 cores (8192 rows each).
"""
import numpy as np

import concourse.bass as bass
import concourse.bacc as bacc
import concourse.tile as tile
import concourse.mybir as mybir
from concourse import bass_utils
from concourse.masks import make_identity

F32 = mybir.dt.float32
F16 = mybir.dt.float16
BF16 = mybir.dt.bfloat16
FP8 = mybir.dt.float8e4
GT = mybir.AluOpType.is_gt
MUL = mybir.AluOpType.mult
ADD = mybir.AluOpType.add
SUB = mybir.AluOpType.subtract
DR = mybir.MatmulPerfMode.DoubleRow
IDENT = mybir.ActivationFunctionType.Identity

N_CORES = 8
B = 65536
IND, HID, OUT = 768, 512, 10
B_LOC = B // N_CORES          # 8192
BC = 512                      # batch chunk
NBC = B_LOC // BC             # 16
KC = IND // 128               # 6 k-chunks
KP = KC // 2                  # 3 k-pairs
HC = HID // 128               # 4 hid-chunks
AGA_CH = 12                   # chunks covered by the early AllGather
EPS = 1e-5

_cache = {}


def build():
    if "nc" in _cache:
        return _cache["nc"]
    nc = bacc.Bacc("TRN2", target_bir_lowering=False, debug=False,
                   num_devices=N_CORES)
    x = nc.dram_tensor("x", [B_LOC, IND], F32, kind="ExternalInput")
    w1 = nc.dram_tensor("w1", [HID, IND], F32, kind="ExternalInput")
    w2 = nc.dram_tensor("w2", [OUT, HID], F32, kind="ExternalInput")
    g1 = nc.dram_tensor("g1", [HID], F32, kind="ExternalInput")
    b1 = nc.dram_tensor("b1", [HID], F32, kind="ExternalInput")
    g2 = nc.dram_tensor("g2", [OUT], F32, kind="ExternalInput")
    b2 = nc.dram_tensor("b2", [OUT], F32, kind="ExternalInput")
    o_out = nc.dram_tensor("o_out", [OUT, B_LOC], F32, kind="ExternalOutput")

    with tile.TileContext(nc) as tc:
        with (
            tc.tile_pool(name="cst", bufs=1) as cst,
            tc.tile_pool(name="stage", bufs=3) as stage,
            tc.tile_pool(name="psx", bufs=2, space="PSUM") as psx,
            tc.tile_pool(name="psmm", bufs=1, space="PSUM") as psmm,
            tc.tile_pool(name="dram", bufs=1, space="DRAM") as dpool,
        ):
            ident = cst.tile([128, 128], F32)
            make_identity(nc, ident[:])
            wup_sb = cst.tile([128, 1], F32)

            # warm-up collective: pays the ncfw cold-start (init barrier +
            # first-gather) during phase A.  gpsimd queue is otherwise empty
            # in v2, so the doorbell fires ~immediately.
            wloc = dpool.tile([128, 1], F32)
            wgat = dpool.tile([128 * N_CORES, 1], F32)
            with tc.high_priority():
                nc.vector.memset(wup_sb[:], 0.0)
                nc.sync.dma_start(out=wloc[:], in_=wup_sb[:])
                nc.gpsimd.collective_compute(
                    "AllGather", mybir.AluOpType.bypass,
                    ins=[wloc.opt()], outs=[wgat.opt()],
                    replica_groups=[list(range(N_CORES))])

            # prefetch x chunk 0 before weight prep grabs the sync queue
            xf_first = stage.tile([128, 4, IND], F32, tag="xf")
            nc.sync.dma_start(
                out=xf_first[:],
                in_=x.ap()[0:BC].rearrange("(s p) f -> p s f", p=128))

            # ---------------- weight prep (one-time, tiny) ----------------
            # W1 [512, 768] -> [128 part(hid%128), 4 hid-chunk, 768] fp32
            w1f = cst.tile([128, HC, IND], F32)
            nc.sync.dma_start(out=w1f[:], in_=w1.ap().rearrange("(c p) f -> p c f", p=128))
            # sign(W1).T fp8: k-pairs 1,2 (k=2..5) are +-1 and pair with
            # ACT-sign x codes; pair 0 (k=0,1) is +-2 and pairs with DVE
            # is_gt {0,1} x codes.
            w1sT = cst.tile([128, KC, HID], FP8)
            for k in range(KC):
                pw = psmm.tile([128, HC, 128], F32, tag=f"mm{k % 4}")
                for c in range(HC):
                    nc.tensor.transpose(pw[:, c, :], w1f[:, c, k * 128:(k + 1) * 128], ident[:])
                pwv = pw[:].rearrange("p c f -> p (c f)")
                nc.scalar.sign(w1sT[:, k, :], pwv)
                if k < 2:
                    nc.scalar.mul(w1sT[:, k, :], w1sT[:, k, :], 2.0)

            # ---------------- persistent big buffers ----------------
            xT8 = cst.tile([128, KC, B_LOC], FP8, tag="big8")   # 48 KB/part
            h1s = cst.tile([128, HC, B_LOC], F16, tag="hbig")   # 64 KB/part
            st1 = cst.tile([128, HC, NBC * 6], F32)             # per-chunk stats
            st2 = cst.tile([OUT, NBC * 6], F32)

            # ---------------- phase A: x -> codes -> mm1 -> h1 ----------------
            # software-pipelined by one chunk: PE order per cycle is
            # [mm1(c-1)][transposes(c)] so mm1 never waits on fresh evacs.
            xfs = {0: xf_first}

            def emit_transposes(c):
                xf = xfs.pop(c)
                half = xf[:, :, :]
                cs = c * BC
                for kp in range(KP):
                    pt = psx.tile([128, 2, 4, 128], F32, tag="pt")
                    for j in range(2):
                        k = 2 * kp + j
                        for s in range(4):
                            nc.tensor.transpose(pt[:, j, s, :], half[:, s, k * 128:(k + 1) * 128], ident[:])
                    ptv = pt[:].rearrange("p j s f -> p (j s f)")
                    dst = xT8[:, 2 * kp:2 * kp + 2, cs:cs + BC]
                    if kp == 0:
                        nc.vector.tensor_scalar(dst, ptv.rearrange("p (j f) -> p j f", j=2), 0.0, None, GT)
                    else:
                        nc.scalar.sign(dst, ptv.rearrange("p (j f) -> p j f", j=2))

            def emit_mm1(c):
                cs = c * BC
                for h in range(HC):
                    mp = psmm.tile([128, BC], F32, tag=f"mm{h}")
                    for k2 in range(KP):
                        nc.tensor.matmul(
                            mp[:],
                            w1sT[:, 2 * k2:2 * k2 + 2, h * 128:(h + 1) * 128],
                            xT8[:, 2 * k2:2 * k2 + 2, cs:cs + BC],
                            start=(k2 == 0), stop=(k2 == KP - 1),
                            perf_mode=DR)
                    if h == HC - 1:
                        nc.vector.tensor_copy(h1s[:, h, cs:cs + BC], mp[:])
                    else:
                        nc.scalar.copy(h1s[:, h, cs:cs + BC], mp[:])

            def emit_stats(c):
                cs = c * BC
                for h in range(HC):
                    nc.vector.bn_stats(st1[:, h, c * 6:(c + 1) * 6],
                                       h1s[:, h, cs:cs + BC])

            ag_bufs = {}

            def emit_ag1(tag, ch_lo, ch_hi, weight):
                # aggregate chunks [ch_lo, ch_hi) -> prescaled (m, E[x2])
                # payload so the cross-core combine is a plain sum.
                agg = cst.tile([128, HC, 2], F32, tag=f"agg{tag}")
                for h in range(HC):
                    nc.vector.bn_aggr(
                        agg[:, h, :],
                        st1[:, h, ch_lo * 6:ch_hi * 6].rearrange("p (n s) -> p n s", s=6))
                q = cst.tile([128, HC], F32, tag=f"q{tag}")
                nc.vector.tensor_tensor(q[:], agg[:, :, 0], agg[:, :, 0], MUL)
                nc.vector.tensor_tensor(q[:], q[:], agg[:, :, 1], ADD)   # E[x^2]
                pay = cst.tile([128, HC, 2], F32, tag=f"pay{tag}")
                nc.vector.tensor_scalar(pay[:, :, 0], agg[:, :, 0], weight, None, MUL)
                nc.vector.tensor_scalar(pay[:, :, 1], q[:], weight, None, MUL)
                loc = dpool.tile([128, HC * 2], F32)
                gat = dpool.tile([128 * N_CORES, HC * 2], F32)
                nc.gpsimd.dma_start(out=loc[:], in_=pay[:].rearrange("p c s -> p (c s)"))
                nc.gpsimd.collective_compute(
                    "AllGather", mybir.AluOpType.bypass,
                    ins=[loc.opt()], outs=[gat.opt()],
                    replica_groups=[list(range(N_CORES))])
                ga = cst.tile([128, N_CORES, HC * 2], F32, tag=f"ga{tag}")
                nc.sync.dma_start(out=ga[:], in_=gat[:].rearrange("(c p) s -> p c s", p=128))
                ag_bufs[tag] = ga

            with nc.named_scope("phaseA"):
                for c in range(NBC):
                    if c + 1 < NBC:
                        xf = stage.tile([128, 4, IND], F32, tag="xf")
                        bs = (c + 1) * BC
                        nc.sync.dma_start(
                            out=xf[:],
                            in_=x.ap()[bs:bs + BC].rearrange("(s p) f -> p s f", p=128))
                        xfs[c + 1] = xf
                    if c > 0:
                        emit_mm1(c - 1)
                        emit_stats(c - 1)
                    emit_transposes(c)
                    if c == AGA_CH + 1:
                        emit_ag1("a", 0, AGA_CH, AGA_CH / NBC)
                emit_mm1(NBC - 1)
                emit_stats(NBC - 1)
            emit_ag1("b", AGA_CH, NBC, (NBC - AGA_CH) / NBC)

            # per-partition copies of g/b vectors
            g1c = cst.tile([128, HC], F32)
            b1c = cst.tile([128, HC], F32)
            for c in range(HC):
                nc.sync.dma_start(out=g1c[:, c:c + 1], in_=g1.ap()[c * 128:(c + 1) * 128])
                nc.sync.dma_start(out=b1c[:, c:c + 1], in_=b1.ap()[c * 128:(c + 1) * 128])
            g2c = cst.tile([OUT, 1], F32)
            b2c = cst.tile([OUT, 1], F32)
            nc.sync.dma_start(out=g2c[:], in_=g2.ap())
            nc.sync.dma_start(out=b2c[:], in_=b2.ap())

            # W2 prep: k=0,1 pair with DVE {0,1} codes -> bf16 +-2;
            # k=2,3 pair with ACT sign codes -> fp8 +-1.
            w2f = cst.tile([OUT, HID], F32)
            nc.sync.dma_start(out=w2f[:], in_=w2.ap())
            w2sTb = cst.tile([128, 2, 16], BF16)
            w2sT8 = cst.tile([128, 2, 16], FP8)
            nc.vector.memset(w2sTb[:], 0.0)
            nc.vector.memset(w2sT8[:], 0.0)
            for c in range(HC):
                pw2 = psmm.tile([128, OUT], F32, tag="mm1")
                nc.tensor.transpose(pw2[:], w2f[:, c * 128:(c + 1) * 128], ident[:OUT, :OUT])
                if c < 2:
                    nc.scalar.sign(w2sTb[:, c, 0:OUT], pw2[:])
                    nc.scalar.mul(w2sTb[:, c, 0:OUT], w2sTb[:, c, 0:OUT], 2.0)
                else:
                    nc.scalar.sign(w2sT8[:, c - 2, 0:OUT], pw2[:])

            # ---------------- combine stats -> thresholds ----------------
            with nc.named_scope("combine1"):
                ga_a, ga_b = ag_bufs["a"], ag_bufs["b"]
                suma = cst.tile([128, HC * 2], F32)
                sumb = cst.tile([128, HC * 2], F32)
                nc.vector.tensor_reduce(suma[:], ga_a[:].rearrange("p c s -> p s c"),
                                        mybir.AxisListType.X, ADD)
                nc.vector.tensor_reduce(sumb[:], ga_b[:].rearrange("p c s -> p s c"),
                                        mybir.AxisListType.X, ADD)
                tot = cst.tile([128, HC, 2], F32)
                nc.vector.tensor_tensor(tot[:].rearrange("p c s -> p (c s)"), suma[:], sumb[:], ADD)
                m1 = cst.tile([128, HC], F32)
                e2 = cst.tile([128, HC], F32)
                nc.vector.tensor_scalar(m1[:], tot[:, :, 0], 1.0 / N_CORES, None, MUL)
                nc.vector.tensor_scalar(e2[:], tot[:, :, 1], 1.0 / N_CORES, None, MUL)
                m1sq = cst.tile([128, HC], F32)
                nc.vector.tensor_tensor(m1sq[:], m1[:], m1[:], MUL)
                v1 = cst.tile([128, HC], F32)
                nc.vector.tensor_tensor(v1[:], e2[:], m1sq[:], SUB)
                sd1 = cst.tile([128, HC], F32)
                nc.vector.tensor_scalar(sd1[:], v1[:], 1.0, EPS, MUL, ADD)
                nc.scalar.sqrt(sd1[:], sd1[:])
                ig1 = cst.tile([128, HC], F32)
                nc.vector.reciprocal(ig1[:], g1c[:])
                corr = cst.tile([128, HC], F32)
                nc.vector.tensor_tensor(corr[:], b1c[:], ig1[:], MUL)
                nc.vector.tensor_tensor(corr[:], corr[:], sd1[:], MUL)
                posT = cst.tile([128, HC], F32)
                negT = cst.tile([128, HC], F32)
                nc.vector.tensor_tensor(posT[:], m1[:], corr[:], SUB)
                nc.vector.tensor_scalar(negT[:], posT[:], -1.0, None, MUL)

            # ---------------- phase B: sign -> mm2 -> h2 ----------------
            h2T = cst.tile([OUT, B_LOC], F32, tag="big8")  # reuses xT8 slot
            with nc.named_scope("phaseB"):
                for g in range(NBC // 2):      # 2-chunk slabs
                    gs = g * 2 * BC
                    scb = stage.tile([128, 2, 2 * BC], BF16, tag="scb")
                    sc8 = stage.tile([128, 2, 2 * BC], FP8, tag="sc8")
                    for h in range(2):         # DVE {0,1} codes, bf16 (2x mode)
                        nc.vector.tensor_scalar(scb[:, h, :], h1s[:, h, gs:gs + 2 * BC],
                                                posT[:, h:h + 1], None, GT)
                    for h in range(2, HC):     # ACT sign codes, fp8 +-1
                        nc.scalar.sign(sc8[:, h - 2, :], h1s[:, h, gs:gs + 2 * BC],
                                       bias=negT[:, h:h + 1])
                    for c2 in range(2):
                        c = 2 * g + c2
                        cs2 = c2 * BC
                        mp2 = psmm.tile([16, BC], F32, tag=f"mm{c % 4}")
                        nc.tensor.matmul(mp2[:], w2sTb[:, 0, :], scb[:, 0, cs2:cs2 + BC],
                                         start=True, stop=False, skip_group_check=True)
                        nc.tensor.matmul(mp2[:], w2sTb[:, 1, :], scb[:, 1, cs2:cs2 + BC],
                                         start=False, stop=False, skip_group_check=True)
                        nc.tensor.matmul(mp2[:], w2sT8[:, :, :], sc8[:, :, cs2:cs2 + BC],
                                         start=False, stop=True, perf_mode=DR,
                                         skip_group_check=True)
                        nc.scalar.copy(h2T[:, c * BC:(c + 1) * BC], mp2[:OUT, :])
                        nc.vector.bn_stats(st2[:, c * 6:(c + 1) * 6],
                                           h2T[:, c * BC:(c + 1) * BC])

            agg2 = cst.tile([OUT, 2], F32)
            nc.vector.bn_aggr(agg2[:], st2[:].rearrange("p (n s) -> p n s", s=6))

            # ---------------- AllGather 2 ----------------
            loc2 = dpool.tile([OUT, 2], F32)
            gat2 = dpool.tile([OUT * N_CORES, 2], F32)
            nc.gpsimd.dma_start(out=loc2[:], in_=agg2[:])
            nc.gpsimd.collective_compute(
                "AllGather", mybir.AluOpType.bypass,
                ins=[loc2.opt()], outs=[gat2.opt()],
                replica_groups=[list(range(N_CORES))])
            ga2 = cst.tile([OUT, N_CORES, 2], F32)
            nc.sync.dma_start(out=ga2[:], in_=gat2[:].rearrange("(c p) s -> p c s", p=OUT))

            with nc.named_scope("combine2"):
                q2 = cst.tile([OUT, N_CORES, 2], F32)
                nc.vector.tensor_tensor(q2[:], ga2[:], ga2[:], MUL)
                msum2 = cst.tile([OUT, 2], F32)
                qsum2 = cst.tile([OUT, 2], F32)
                nc.vector.tensor_reduce(msum2[:], ga2[:].rearrange("p c s -> p s c"),
                                        mybir.AxisListType.X, ADD)
                nc.vector.tensor_reduce(qsum2[:], q2[:].rearrange("p c s -> p s c"),
                                        mybir.AxisListType.X, ADD)
                m2 = cst.tile([OUT, 1], F32)
                nc.vector.tensor_scalar(m2[:], msum2[:, 0:1], 1.0 / N_CORES, None, MUL)
                e22 = cst.tile([OUT, 1], F32)
                nc.vector.tensor_tensor(e22[:], qsum2[:, 0:1], msum2[:, 1:2], ADD)
                nc.vector.tensor_scalar(e22[:], e22[:], 1.0 / N_CORES, None, MUL)
                m2sq = cst.tile([OUT, 1], F32)
                nc.vector.tensor_tensor(m2sq[:], m2[:], m2[:], MUL)
                v2 = cst.tile([OUT, 1], F32)
                nc.vector.tensor_tensor(v2[:], e22[:], m2sq[:], SUB)
                sd2 = cst.tile([OUT, 1], F32)
                nc.vector.tensor_scalar(sd2[:], v2[:], 1.0, EPS, MUL, ADD)
                nc.scalar.sqrt(sd2[:], sd2[:])
                r2 = cst.tile([OUT, 1], F32)
                nc.vector.reciprocal(r2[:], sd2[:])
                scale2 = cst.tile([OUT, 1], F32)
                nc.vector.tensor_tensor(scale2[:], r2[:], g2c[:], MUL)
                shift2 = cst.tile([OUT, 1], F32)
                nc.vector.tensor_tensor(shift2[:], m2[:], scale2[:], MUL)
                nc.vector.tensor_tensor(shift2[:], b2c[:], shift2[:], SUB)

            # final affine (in place), store transposed; host undoes transpose
            for sl in range(4):
                ss = sl * (B_LOC // 4)
                se = ss + B_LOC // 4
                nc.vector.tensor_scalar(h2T[:, ss:se], h2T[:, ss:se], scale2[:], shift2[:], MUL, ADD)
                nc.sync.dma_start(out=o_out.ap()[:, ss:se], in_=h2T[:, ss:se])

    nc.compile()
    _cache["nc"] = nc
    return nc


def kernel(x, W1, W2, g1, b1, g2, b2, _trace=False, _trace_cores=None):
    nc = build()
    x = np.ascontiguousarray(np.asarray(x, dtype=np.float32))
    in_maps = []
    for c in range(N_CORES):
        in_maps.append({
            "x": x[c * B_LOC:(c + 1) * B_LOC],
            "w1": np.asarray(W1, np.float32),
            "w2": np.asarray(W2, np.float32),
            "g1": np.asarray(g1, np.float32),
            "b1": np.asarray(b1, np.float32),
            "g2": np.asarray(g2, np.float32),
            "b2": np.asarray(b2, np.float32),
        })
    kwargs = {}
    if _trace_cores is not None:
        kwargs["trace_cores"] = _trace_cores
    res = bass_utils.run_bass_kernel_spmd(nc, in_maps, core_ids=list(range(N_CORES)),
                                          trace=_trace, **kwargs)
    out = np.concatenate([np.ascontiguousarray(r["o_out"].T) for r in res.results], axis=0)
    if _trace:
        kernel.last_results = res
    return out


# revision 5
# speedup vs baseline: 5.6937x; 5.6937x over previous
"""Trainium2 Bass kernel for the binarized 2-layer MLP (nn_FC_small), v3.

Network (reference semantics):
    h  = sign(x) @ sign(W1).T            # [B, 512], B = 65536, in = 768
    h  = batchnorm(h, g1, b1)            # training-mode, full-batch stats
    h  = clip(h, -1, 1)                  # hardtanh (sign-preserving)
    o  = sign(h) @ sign(W2).T            # [B, 10]
    o  = batchnorm(o, g2, b2)

Identities: sign(clip(z)) == sign(z); with g>0,
sign((h-mu)*r*g + b) == sign(h - T), T = mu - b/(r*g).  Per-k-chunk code
conventions ({0,1} via DVE is_gt with +-2 weights vs {-1,1} via ACT sign
with +-1 weights) fold into the stationary scaling; the batch-independent
column shift is absorbed by the batch statistics.

v3 (trace-driven):
  * x via HWDGE fp32 (SWDGE cast-DMA caps at ~half HBM rate); PE
    transposes in f32r via bitcast (1-pass, ~56 ns vs fp32's 2-pass
    LOW_HIGH ~109 ns).
  * 1024-row super-chunks: elementwise evacs/copies run on [128, 1024]
    tiles to amortize the per-op fixed costs (ACT ~352 cyc, DVE ~120).
  * AllGather payloads PE-transposed to [8, 128]: 128-partition bounce
    buffers cost ~15 us in tiny SWDGE descriptors, few-partition ~6 us.
  * stats AllGather split 12/4 chunks so the big gather hides under
    phase A's tail.
  * phase B: DVE {0,1}-bf16 codes (2x mode) + ACT sign-fp8 codes, mixed
    bf16 + fp8-DoubleRow mm2 into one PSUM group.
"""
import numpy as np

import concourse.bass as bass
import concourse.bacc as bacc
import concourse.tile as tile
import concourse.mybir as mybir
from concourse import bass_utils
from concourse.masks import make_identity

F32 = mybir.dt.float32
F32R = mybir.dt.float32r
F16 = mybir.dt.float16
BF16 = mybir.dt.bfloat16
FP8 = mybir.dt.float8e4
GT = mybir.AluOpType.is_gt
MUL = mybir.AluOpType.mult
ADD = mybir.AluOpType.add
SUB = mybir.AluOpType.subtract
DR = mybir.MatmulPerfMode.DoubleRow

N_CORES = 8
B = 65536
IND, HID, OUT = 768, 512, 10
B_LOC = B // N_CORES          # 8192
BC = 512                      # batch chunk (psum bank / bn_stats grain)
NBC = B_LOC // BC             # 16
SC = 1024                     # super-chunk (elementwise/DMA grain)
NSC = B_LOC // SC             # 8
KC = IND // 128               # 6 k-chunks
HC = HID // 128               # 4 hid-chunks
AGA_CH = 12                   # chunks covered by the early AllGather
EPS = 1e-5
USE_F32R_T = True

_cache = {}


def build():
    if "nc" in _cache:
        return _cache["nc"]
    nc = bacc.Bacc("TRN2", target_bir_lowering=False, debug=False,
                   num_devices=N_CORES)
    x = nc.dram_tensor("x", [B_LOC, IND], F32, kind="ExternalInput")
    w1 = nc.dram_tensor("w1", [HID, IND], F32, kind="ExternalInput")
    w2 = nc.dram_tensor("w2", [OUT, HID], F32, kind="ExternalInput")
    g1 = nc.dram_tensor("g1", [HID], F32, kind="ExternalInput")
    b1 = nc.dram_tensor("b1", [HID], F32, kind="ExternalInput")
    g2 = nc.dram_tensor("g2", [OUT], F32, kind="ExternalInput")
    b2 = nc.dram_tensor("b2", [OUT], F32, kind="ExternalInput")
    o_out = nc.dram_tensor("o_out", [OUT, B_LOC], F32, kind="ExternalOutput")

    TDT = F32R if USE_F32R_T else F32

    def tcast(ap):
        return ap.bitcast(F32R) if USE_F32R_T else ap

    with tile.TileContext(nc) as tc:
        with (
            tc.tile_pool(name="cst", bufs=1) as cst,
            tc.tile_pool(name="stage", bufs=3) as stage,
            tc.tile_pool(name="psx", bufs=2, space="PSUM") as psx,
            tc.tile_pool(name="psmm", bufs=1, space="PSUM") as psmm,
            tc.tile_pool(name="dram", bufs=1, space="DRAM") as dpool,
        ):
            identr = cst.tile([128, 128], TDT)
            make_identity(nc, identr[:])
            identf = cst.tile([128, 128], F32)
            make_identity(nc, identf[:])
            wup_sb = cst.tile([128, 1], F32)

            # warm-up collective: pays ncfw cold-start during phase A
            wloc = dpool.tile([128, 1], F32)
            wgat = dpool.tile([128 * N_CORES, 1], F32)
            with tc.high_priority():
                nc.vector.memset(wup_sb[:], 0.0)
                nc.sync.dma_start(out=wloc[:], in_=wup_sb[:])
                nc.gpsimd.collective_compute(
                    "AllGather", mybir.AluOpType.bypass,
                    ins=[wloc.opt()], outs=[wgat.opt()],
                    replica_groups=[list(range(N_CORES))])

            # prefetch first x chunks on the sync (HWDGE) queue
            xfs = {}
            for c in range(2):
                xf = stage.tile([128, 4, IND], F32, tag="xf")
                nc.sync.dma_start(
                    out=xf[:],
                    in_=x.ap()[c * BC:(c + 1) * BC].rearrange("(s p) f -> p s f", p=128))
                xfs[c] = xf

            # ---------------- weight prep (one-time, via SWDGE queue) ----
            w1f = cst.tile([128, HC, IND], F32)
            nc.gpsimd.dma_start(out=w1f[:], in_=w1.ap().rearrange("(c p) f -> p c f", p=128))
            # sign(W1).T fp8: k<3 +-2 (pairs with DVE {0,1} codes),
            # k>=3 +-1 (pairs with ACT sign codes)
            w1sT = cst.tile([128, KC, HID], FP8)
            for k in range(KC):
                pw = psmm.tile([128, HC, 128], TDT, tag=f"mm{k % 2}")
                for c in range(HC):
                    nc.tensor.transpose(pw[:, c, :], tcast(w1f[:, c, k * 128:(k + 1) * 128]), identr[:])
                pwv = pw[:].rearrange("p c f -> p (c f)")
                nc.scalar.sign(w1sT[:, k, :], pwv)
                if k < 3:
                    nc.scalar.mul(w1sT[:, k, :], w1sT[:, k, :], 2.0)

            # ---------------- persistent big buffers ----------------
            xT8 = cst.tile([128, KC, B_LOC], FP8, tag="big8")   # 48 KB/part
            h1s = cst.tile([128, HC, B_LOC], F16, tag="hbig")   # 64 KB/part
            st1 = cst.tile([128, HC, NBC * 6], F32)

            # ---------------- phase A ----------------
            # per super-chunk (1024 rows): 48 transposes -> 6 wide code
            # evacs -> 24 DR matmuls (2 rotating 2-bank psum tiles) ->
            # 4 wide h1 copies -> 8 bn_stats.  PE order per cycle is
            # [mm1(sc-1)][transposes(sc)] so mm1 never waits on evacs.
            def emit_transposes(sc):
                xa, xb = xfs.pop(2 * sc), xfs.pop(2 * sc + 1)
                cs = sc * SC
                for k in range(KC):
                    pt = psx.tile([128, 8, 128], TDT, tag="pt")
                    for s in range(4):
                        nc.tensor.transpose(pt[:, s, :], tcast(xa[:, s, k * 128:(k + 1) * 128]), identr[:])
                    for s in range(4):
                        nc.tensor.transpose(pt[:, 4 + s, :], tcast(xb[:, s, k * 128:(k + 1) * 128]), identr[:])
                    ptv = pt[:].rearrange("p s f -> p (s f)")
                    if k < 3:
                        nc.vector.tensor_scalar(xT8[:, k, cs:cs + SC], ptv, 0.0, None, GT)
                    else:
                        nc.scalar.sign(xT8[:, k, cs:cs + SC], ptv)

            def emit_mm1(sc):
                cs = sc * SC
                for h in range(HC):
                    mp = psmm.tile([128, 2, BC], F32, tag=f"mm{h % 2}")
                    for c2 in range(2):
                        for k2 in range(KC // 2):
                            nc.tensor.matmul(
                                mp[:, c2, :],
                                w1sT[:, 2 * k2:2 * k2 + 2, h * 128:(h + 1) * 128],
                                xT8[:, 2 * k2:2 * k2 + 2, cs + c2 * BC:cs + (c2 + 1) * BC],
                                start=(k2 == 0), stop=(k2 == KC // 2 - 1),
                                perf_mode=DR)
                    nc.scalar.copy(h1s[:, h, cs:cs + SC], mp[:].rearrange("p c f -> p (c f)"))
                    for c2 in range(2):
                        c = 2 * sc + c2
                        nc.vector.bn_stats(st1[:, h, c * 6:(c + 1) * 6],
                                           h1s[:, h, c * BC:(c + 1) * BC])

            ag_bufs = {}

            def emit_ag1(tag, ch_lo, ch_hi, weight):
                # chunk range -> (w*mean, w*E[x^2]) payload, PE-transposed
                # to [8, 128] so the collective bounce uses fat descriptors.
                agg = cst.tile([128, HC, 2], F32, tag=f"agg{tag}")
                for h in range(HC):
                    nc.vector.bn_aggr(
                        agg[:, h, :],
                        st1[:, h, ch_lo * 6:ch_hi * 6].rearrange("p (n s) -> p n s", s=6))
                q = cst.tile([128, HC], F32, tag=f"q{tag}")
                nc.vector.tensor_tensor(q[:], agg[:, :, 0], agg[:, :, 0], MUL)
                nc.vector.tensor_tensor(q[:], q[:], agg[:, :, 1], ADD)   # E[x^2]
                pay = cst.tile([128, HC, 2], F32, tag=f"pay{tag}")
                nc.vector.tensor_scalar(pay[:, :, 0], agg[:, :, 0], weight, None, MUL)
                nc.vector.tensor_scalar(pay[:, :, 1], q[:], weight, None, MUL)
                pp = psmm.tile([8, 128], F32, tag="mm0")
                nc.tensor.transpose(pp[:], pay[:].rearrange("p c s -> p (c s)"), identf[:8, :8])
                payT = cst.tile([8, 128], F32, tag=f"payT{tag}")
                nc.vector.tensor_copy(payT[:], pp[:])
                loc = dpool.tile([8, 128], F32)
                gat = dpool.tile([8 * N_CORES, 128], F32)
                nc.gpsimd.dma_start(out=loc[:], in_=payT[:])
                nc.gpsimd.collective_compute(
                    "AllGather", mybir.AluOpType.bypass,
                    ins=[loc.opt()], outs=[gat.opt()],
                    replica_groups=[list(range(N_CORES))])
                ga = cst.tile([8, N_CORES, 128], F32, tag=f"ga{tag}")
                nc.sync.dma_start(out=ga[:], in_=gat[:].rearrange("(c s) f -> s c f", s=8))
                ag_bufs[tag] = ga

            with nc.named_scope("phaseA"):
                for sc in range(NSC):
                    for c in (2 * sc + 2, 2 * sc + 3):
                        if c < NBC:
                            xf = stage.tile([128, 4, IND], F32, tag="xf")
                            nc.sync.dma_start(
                                out=xf[:],
                                in_=x.ap()[c * BC:(c + 1) * BC].rearrange("(s p) f -> p s f", p=128))
                            xfs[c] = xf
                    if sc > 0:
                        emit_mm1(sc - 1)
                    emit_transposes(sc)
                    if sc == AGA_CH // 2 + 1:
                        emit_ag1("a", 0, AGA_CH, AGA_CH / NBC)
                emit_mm1(NSC - 1)
            emit_ag1("b", AGA_CH, NBC, (NBC - AGA_CH) / NBC)

            # per-partition copies of g/b vectors (SWDGE queue, tiny)
            g1c = cst.tile([128, HC], F32)
            b1c = cst.tile([128, HC], F32)
            for c in range(HC):
                nc.gpsimd.dma_start(out=g1c[:, c:c + 1], in_=g1.ap()[c * 128:(c + 1) * 128])
                nc.gpsimd.dma_start(out=b1c[:, c:c + 1], in_=b1.ap()[c * 128:(c + 1) * 128])
            g2c = cst.tile([OUT, 1], F32)
            b2c = cst.tile([OUT, 1], F32)
            nc.gpsimd.dma_start(out=g2c[:], in_=g2.ap())
            nc.gpsimd.dma_start(out=b2c[:], in_=b2.ap())

            # W2 prep: k<2 -> bf16 +-2 (DVE codes); k>=2 -> fp8 +-1 (ACT)
            w2f = cst.tile([OUT, HID], F32)
            nc.gpsimd.dma_start(out=w2f[:], in_=w2.ap())
            w2sTb = cst.tile([128, 2, 16], BF16)
            w2sT8 = cst.tile([128, 2, 16], FP8)
            nc.vector.memset(w2sTb[:], 0.0)
            nc.vector.memset(w2sT8[:], 0.0)
            for c in range(HC):
                pw2 = psmm.tile([128, OUT], F32, tag="mm1")
                nc.tensor.transpose(pw2[:], w2f[:, c * 128:(c + 1) * 128], identf[:OUT, :OUT])
                if c < 2:
                    nc.scalar.sign(w2sTb[:, c, 0:OUT], pw2[:])
                    nc.scalar.mul(w2sTb[:, c, 0:OUT], w2sTb[:, c, 0:OUT], 2.0)
                else:
                    nc.scalar.sign(w2sT8[:, c - 2, 0:OUT], pw2[:])

            # ---------------- combine stats -> thresholds ----------------
            with nc.named_scope("combine1"):
                ga_a, ga_b = ag_bufs["a"], ag_bufs["b"]
                suma = cst.tile([8, 128], F32)
                sumb = cst.tile([8, 128], F32)
                nc.vector.tensor_reduce(suma[:], ga_a[:].rearrange("s c f -> s f c"),
                                        mybir.AxisListType.X, ADD)
                nc.vector.tensor_reduce(sumb[:], ga_b[:].rearrange("s c f -> s f c"),
                                        mybir.AxisListType.X, ADD)
                tot8 = cst.tile([8, 128], F32)
                nc.vector.tensor_tensor(tot8[:], suma[:], sumb[:], ADD)
                pb = psmm.tile([128, 8], F32, tag="mm0")
                nc.tensor.transpose(pb[:], tot8[:], identf[:8, :8])
                tot = cst.tile([128, HC, 2], F32)
                nc.vector.tensor_copy(tot[:].rearrange("p c s -> p (c s)"), pb[:])
                m1 = cst.tile([128, HC], F32)
                e2 = cst.tile([128, HC], F32)
                nc.vector.tensor_scalar(m1[:], tot[:, :, 0], 1.0 / N_CORES, None, MUL)
                nc.vector.tensor_scalar(e2[:], tot[:, :, 1], 1.0 / N_CORES, None, MUL)
                m1sq = cst.tile([128, HC], F32)
                nc.vector.tensor_tensor(m1sq[:], m1[:], m1[:], MUL)
                v1 = cst.tile([128, HC], F32)
                nc.vector.tensor_tensor(v1[:], e2[:], m1sq[:], SUB)
                sd1 = cst.tile([128, HC], F32)
                nc.vector.tensor_scalar(sd1[:], v1[:], 1.0, EPS, MUL, ADD)
                nc.scalar.sqrt(sd1[:], sd1[:])
                ig1 = cst.tile([128, HC], F32)
                nc.vector.reciprocal(ig1[:], g1c[:])
                corr = cst.tile([128, HC], F32)
                nc.vector.tensor_tensor(corr[:], b1c[:], ig1[:], MUL)
                nc.vector.tensor_tensor(corr[:], corr[:], sd1[:], MUL)
                posT = cst.tile([128, HC], F32)
                negT = cst.tile([128, HC], F32)
                nc.vector.tensor_tensor(posT[:], m1[:], corr[:], SUB)
                nc.vector.tensor_scalar(negT[:], posT[:], -1.0, None, MUL)

            # ---------------- phase B ----------------
            h2T = cst.tile([OUT, B_LOC], F32, tag="big8")  # reuses xT8 slot
            st2 = cst.tile([OUT, NBC * 6], F32)

            def emit_codes(g):
                gs = g * SC
                scb = stage.tile([128, 2, SC], BF16, tag="scb")
                sc8 = stage.tile([128, 2, SC], FP8, tag="sc8")
                for h in range(2):
                    nc.vector.tensor_scalar(scb[:, h, :], h1s[:, h, gs:gs + SC],
                                            posT[:, h:h + 1], None, GT)
                for h in range(2, HC):
                    nc.scalar.sign(sc8[:, h - 2, :], h1s[:, h, gs:gs + SC],
                                   bias=negT[:, h:h + 1])
                return scb, sc8

            def emit_mm2(g, bufs):
                scb, sc8 = bufs
                for c2 in range(2):
                    c = 2 * g + c2
                    cs2 = c2 * BC
                    mp2 = psmm.tile([16, BC], F32, tag=f"mm{c % 2}")
                    nc.tensor.matmul(mp2[:], w2sTb[:, 0, :], scb[:, 0, cs2:cs2 + BC],
                                     start=True, stop=False, skip_group_check=True)
                    nc.tensor.matmul(mp2[:], w2sTb[:, 1, :], scb[:, 1, cs2:cs2 + BC],
                                     start=False, stop=False, skip_group_check=True)
                    nc.tensor.matmul(mp2[:], w2sT8[:, :, :], sc8[:, :, cs2:cs2 + BC],
                                     start=False, stop=True, perf_mode=DR,
                                     skip_group_check=True)
                    nc.scalar.copy(h2T[:, c * BC:(c + 1) * BC], mp2[:OUT, :])
                    nc.vector.bn_stats(st2[:, c * 6:(c + 1) * 6],
                                       h2T[:, c * BC:(c + 1) * BC])

            with nc.named_scope("phaseB"):
                prev = emit_codes(0)
                for g in range(NSC):
                    cur = prev
                    if g + 1 < NSC:
                        prev = emit_codes(g + 1)
                    emit_mm2(g, cur)

            agg2 = cst.tile([OUT, 2], F32)
            nc.vector.bn_aggr(agg2[:], st2[:].rearrange("p (n s) -> p n s", s=6))

            # ---------------- AllGather 2 ----------------
            loc2 = dpool.tile([OUT, 2], F32)
            gat2 = dpool.tile([OUT * N_CORES, 2], F32)
            nc.gpsimd.dma_start(out=loc2[:], in_=agg2[:])
            nc.gpsimd.collective_compute(
                "AllGather", mybir.AluOpType.bypass,
                ins=[loc2.opt()], outs=[gat2.opt()],
                replica_groups=[list(range(N_CORES))])
            ga2 = cst.tile([OUT, N_CORES, 2], F32)
            nc.sync.dma_start(out=ga2[:], in_=gat2[:].rearrange("(c p) s -> p c s", p=OUT))

            with nc.named_scope("combine2"):
                q2 = cst.tile([OUT, N_CORES, 2], F32)
                nc.vector.tensor_tensor(q2[:], ga2[:], ga2[:], MUL)
                msum2 = cst.tile([OUT, 2], F32)
                qsum2 = cst.tile([OUT, 2], F32)
                nc.vector.tensor_reduce(msum2[:], ga2[:].rearrange("p c s -> p s c"),
                                        mybir.AxisListType.X, ADD)
                nc.vector.tensor_reduce(qsum2[:], q2[:].rearrange("p c s -> p s c"),
                                        mybir.AxisListType.X, ADD)
                m2 = cst.tile([OUT, 1], F32)
                nc.vector.tensor_scalar(m2[:], msum2[:, 0:1], 1.0 / N_CORES, None, MUL)
                e22 = cst.tile([OUT, 1], F32)
                nc.vector.tensor_tensor(e22[:], qsum2[:, 0:1], msum2[:, 1:2], ADD)
                nc.vector.tensor_scalar(e22[:], e22[:], 1.0 / N_CORES, None, MUL)
                m2sq = cst.tile([OUT, 1], F32)
                nc.vector.tensor_tensor(m2sq[:], m2[:], m2[:], MUL)
                v2 = cst.tile([OUT, 1], F32)
                nc.vector.tensor_tensor(v2[:], e22[:], m2sq[:], SUB)
                sd2 = cst.tile([OUT, 1], F32)
                nc.vector.tensor_scalar(sd2[:], v2[:], 1.0, EPS, MUL, ADD)
                nc.scalar.sqrt(sd2[:], sd2[:])
                r2 = cst.tile([OUT, 1], F32)
                nc.vector.reciprocal(r2[:], sd2[:])
                scale2 = cst.tile([OUT, 1], F32)
                nc.vector.tensor_tensor(scale2[:], r2[:], g2c[:], MUL)
                shift2 = cst.tile([OUT, 1], F32)
                nc.vector.tensor_tensor(shift2[:], m2[:], scale2[:], MUL)
                nc.vector.tensor_tensor(shift2[:], b2c[:], shift2[:], SUB)

            # final affine (in place), store transposed; host undoes it
            for sl in range(4):
                ss = sl * (B_LOC // 4)
                se = ss + B_LOC // 4
                nc.vector.tensor_scalar(h2T[:, ss:se], h2T[:, ss:se], scale2[:], shift2[:], MUL, ADD)
                nc.sync.dma_start(out=o_out.ap()[:, ss:se], in_=h2T[:, ss:se])

    nc.compile()
    _cache["nc"] = nc
    return nc


def kernel(x, W1, W2, g1, b1, g2, b2, _trace=False, _trace_cores=None):
    nc = build()
    x = np.ascontiguousarray(np.asarray(x, dtype=np.float32))
    in_maps = []
    for c in range(N_CORES):
        in_maps.append({
            "x": x[c * B_LOC:(c + 1) * B_LOC],
            "w1": np.asarray(W1, np.float32),
            "w2": np.asarray(W2, np.float32),
            "g1": np.asarray(g1, np.float32),
            "b1": np.asarray(b1, np.float32),
            "g2": np.asarray(g2, np.float32),
            "b2": np.asarray(b2, np.float32),
        })
    kwargs = {}
    if _trace_cores is not None:
        kwargs["trace_cores"] = _trace_cores
    res = bass_utils.run_bass_kernel_spmd(nc, in_maps, core_ids=list(range(N_CORES)),
                                          trace=_trace, **kwargs)
    out = np.concatenate([np.ascontiguousarray(r["o_out"].T) for r in res.results], axis=0)
    if _trace:
        kernel.last_results = res
    return out
